# revision 5
# baseline (speedup 1.0000x reference)
"""Trainium2 Bass kernel for the dense RandLA-Net block.

Reference computation (per batch b, point n, K=16 neighbors):
    enc   = [center(3), npos(3), rel(3), dist(1)]            # 10 dims
    rp    = relu(enc @ W_rel + b_rel)                        # 64
    f     = [rp, nfeat]                                      # 128
    att   = softmax_k(f @ W_att)                             # 128
    agg   = sum_k f * att                                    # 128
    out   = relu(agg @ W_glob + b_glob)                      # 128

Sharding: 8 cores = 4 batches x 2 point-halves (8192 points/core).

Data flow per core: a 256-byte token table in SBUF holds, per point,
its 64 feature channels (bf16) at words 0:64 and its position at words
64:67.  Columns are processed in tiles of 512 points x 16 k-slabs; for
each k-slab (chunk of 512 columns) one SBUF-source transpose
dma_gather (512 indices -- the HW descriptor-ring limit for the
transpose path) materializes nfeat on partitions 0:64 and npos on
partitions 64:67.  dist is computed per 4-chunk group from
(npos-center) via selector matmuls accumulating dist^2 into PSUM rows
0:4, one sqrt, and a 4-descriptor DMA back into partition row 67.  rp
is a 4-row matmul [npos;dist] plus a 3-row center matmul folded via
PSUM accumulation (rel is algebraically folded:  Wc*center + Wn*npos +
Wr*(npos-center) = (Wc-Wr)*center + (Wn+Wr)*npos).  The whole pipeline
runs in the "swapped" channel layout f = [nfeat(0:64); rp(64:128)],
handled by a permuted W_att and an un-permuting accumulation identity.
"""

import os
import sys

import numpy as np

sys.path.insert(0, "/opt/trn_rl_repo")

import ml_dtypes

import concourse.bass as bass
import concourse.tile as tile
from concourse import mybir, bacc
from concourse.bass_utils import run_bass_kernel_spmd

F32 = mybir.dt.float32
BF16 = mybir.dt.bfloat16
I16 = mybir.dt.int16
AF = mybir.ActivationFunctionType
OP = mybir.AluOpType
BF = ml_dtypes.bfloat16

B, C_IN, N, K = 4, 64, 16384, 16
D_REL, C_MID, C_OUT = 64, 128, 128
NP = N // 2            # points per core
NT = 16                # tiles (= point blocks of 512)
TC = 512               # points per tile
LT = TC * K            # 8192 cols per tile
NG = 4                 # chunk groups per tile
GC = 4                 # chunks per group


def _build_kernel():
    nc = bacc.Bacc("TRN2", target_bir_lowering=False)

    xtab = nc.dram_tensor("xtab", [128, 16384], BF16, kind="ExternalInput")
    gidx = nc.dram_tensor("gidx", [128, 8192], I16, kind="ExternalInput")
    posc = nc.dram_tensor("posc", [4, NP], BF16, kind="ExternalInput")
    wcen = nc.dram_tensor("wcen", [128, 64], BF16, kind="ExternalInput")
    wnd = nc.dram_tensor("wnd", [128, 64], BF16, kind="ExternalInput")
    seld = nc.dram_tensor("seld", [128, 128], BF16, kind="ExternalInput")
    wattsw = nc.dram_tensor("wattsw", [128, 128], BF16, kind="ExternalInput")
    pswap = nc.dram_tensor("pswap", [128, 128], BF16, kind="ExternalInput")
    wglob = nc.dram_tensor("wglob", [128, 128], BF16, kind="ExternalInput")
    brel = nc.dram_tensor("brel", [128, 1], F32, kind="ExternalInput")
    bglob = nc.dram_tensor("bglob", [128, 1], F32, kind="ExternalInput")
    outp = nc.dram_tensor("outp", [128, NP], F32, kind="ExternalOutput")

    with tile.TileContext(nc) as tc:
        with tc.tile_pool(name="persist", bufs=1) as pp:
            xtab_sb = pp.tile([128, 16384], BF16)
            gidx_sb = pp.tile([128, 8192], I16)
            pos_sb = pp.tile([128, NP], BF16)
            wcen_sb = pp.tile([128, 64], BF16)
            wnd_sb = pp.tile([128, 64], BF16)
            seld_sb = pp.tile([128, 128], BF16)
            wattsw_sb = pp.tile([128, 128], BF16)
            pswap_sb = pp.tile([128, 128], BF16)
            wglob_sb = pp.tile([128, 128], BF16)
            brel_sb = pp.tile([128, 1], F32)
            bglob_sb = pp.tile([128, 1], F32)
            nc.sync.dma_start(out=xtab_sb, in_=xtab.ap())
            nc.sync.dma_start(out=gidx_sb, in_=gidx.ap())
            nc.sync.dma_start(out=pos_sb[64:68, :], in_=posc.ap())
            nc.sync.dma_start(out=wcen_sb, in_=wcen.ap())
            nc.sync.dma_start(out=wnd_sb, in_=wnd.ap())
            nc.sync.dma_start(out=seld_sb, in_=seld.ap())
            nc.sync.dma_start(out=wattsw_sb, in_=wattsw.ap())
            nc.sync.dma_start(out=pswap_sb, in_=pswap.ap())
            nc.sync.dma_start(out=wglob_sb, in_=wglob.ap())
            nc.sync.dma_start(out=brel_sb, in_=brel.ap())
            nc.sync.dma_start(out=bglob_sb, in_=bglob.ap())

            with tc.tile_pool(name="g", bufs=2) as gp, \
                 tc.tile_pool(name="work", bufs=2) as wp, \
                 tc.tile_pool(name="accps", bufs=1, space="PSUM") as psa, \
                 tc.tile_pool(name="mps", bufs=2, space="PSUM") as mps:
                for t in range(NT):
                    tcols = slice(t * TC, (t + 1) * TC)
                    g = gp.tile([128, LT], BF16, tag="g")
                    ps_den = psa.tile([128, 512], F32, tag="den")
                    ps_num = psa.tile([128, 512], F32, tag="num")
                    for gq in range(NG):
                        for u in range(GC):
                            cc = GC * gq + u
                            gs = g[:, cc * 512:(cc + 1) * 512]
                            gout = bass.AP(tensor=gs.tensor, offset=gs.offset,
                                           ap=[[gs.ap[0][0], 128], [512, 1],
                                               [1, 512]])
                            ib = t * 512 + cc * 32
                            nc.gpsimd.dma_gather(
                                out_ap=gout, in_ap=xtab_sb[:, :],
                                idxs_ap=gidx_sb[:, ib:ib + 32],
                                num_idxs=512, num_idxs_reg=512, elem_size=128,
                                transpose=True,
                                sbuf_tokens_per_rank=128,
                                sbuf_free_dim_per_rank=256)
                        gcols = slice(gq * 2048, (gq + 1) * 2048)
                        # rel = npos - center; m2 = rel^2 (in place)
                        cen = pos_sb[64:67, tcols]
                        cen_b = bass.AP(tensor=cen.tensor, offset=cen.offset,
                                        ap=[[cen.ap[0][0], 3], [0, GC],
                                            [1, TC]])
                        s_t = wp.tile([128, 2048], BF16, tag="s")
                        nc.vector.tensor_tensor(
                            out=s_t[64:67, :].rearrange("p (k i) -> p k i",
                                                        i=TC),
                            in0=g[64:67, gcols].rearrange("p (k i) -> p k i",
                                                          i=TC),
                            in1=cen_b, op=OP.subtract)
                        nc.vector.tensor_mul(s_t[64:67, :], s_t[64:67, :],
                                             s_t[64:67, :])
                        # dist^2 per chunk u -> psd row u
                        psd = mps.tile([128, 512], F32, tag="psd")
                        for u in range(GC):
                            nc.tensor.matmul(psd[0:32, :],
                                             seld_sb[64:67,
                                                     32 * u:32 * u + 32],
                                             s_t[64:67,
                                                 u * 512:(u + 1) * 512],
                                             start=(u == 0), stop=(u == 3),
                                             tile_position=(64, 0),
                                             skip_group_check=True)
                        dsbT = wp.tile([128, 512], BF16, tag="dsbT")
                        nc.scalar.activation(out=dsbT[0:4, :],
                                             in_=psd[0:4, :], func=AF.Sqrt)
                        # scatter the 4 dist rows back into g[67, gcols]
                        drow = g[67:68, gcols]
                        drow = bass.AP(tensor=drow.tensor, offset=drow.offset,
                                       ap=[[drow.ap[0][0], 1], [512, GC],
                                           [1, 512]])
                        nc.sync.dma_start(out=drow, in_=dsbT[0:4, :])

                        for u in range(GC):
                            cc = GC * gq + u
                            ccols = slice(cc * 512, (cc + 1) * 512)
                            ps_rp = mps.tile([128, 512], F32, tag="rp")
                            nc.tensor.matmul(ps_rp[64:128, :],
                                             wcen_sb[64:67, :],
                                             pos_sb[64:67, tcols],
                                             start=True, stop=False,
                                             tile_position=(64, 64),
                                             skip_group_check=True)
                            nc.tensor.matmul(ps_rp[64:128, :],
                                             wnd_sb[64:68, :],
                                             g[64:68, ccols],
                                             start=False, stop=True,
                                             tile_position=(64, 64),
                                             skip_group_check=True)
                            if cc % 2 == 0:
                                nc.scalar.activation(out=g[64:128, ccols],
                                                     in_=ps_rp[64:128, :],
                                                     func=AF.Relu,
                                                     bias=brel_sb[64:128, :],
                                                     scale=1.0)
                            else:
                                nc.vector.tensor_scalar(
                                    out=g[64:128, ccols],
                                    in0=ps_rp[64:128, :],
                                    scalar1=brel_sb[64:128, :],
                                    scalar2=0.0,
                                    op0=OP.add, op1=OP.max)
                            ps_s = mps.tile([128, 512], F32, tag="sc")
                            nc.tensor.matmul(ps_s, wattsw_sb, g[:, ccols],
                                             start=True, stop=True)
                            eu = wp.tile([128, 1024], BF16, tag="eu")
                            nc.scalar.activation(out=eu[:, 0:512], in_=ps_s,
                                                 func=AF.Exp)
                            nc.vector.tensor_mul(eu[:, 512:1024], g[:, ccols],
                                                 eu[:, 0:512])
                            nc.tensor.matmul(ps_den, pswap_sb, eu[:, 0:512],
                                             start=(cc == 0), stop=(cc == 15),
                                             skip_group_check=True)
                            nc.tensor.matmul(ps_num, pswap_sb,
                                             eu[:, 512:1024],
                                             start=(cc == 0), stop=(cc == 15),
                                             skip_group_check=True)
                    rcp = wp.tile([128, 512], F32, tag="rcp")
                    nc.vector.reciprocal(rcp, ps_den)
                    agg = wp.tile([128, 512], BF16, tag="agg")
                    nc.vector.tensor_mul(agg, ps_num, rcp)
                    ps_o = psa.tile([128, 512], F32, tag="den")
                    nc.tensor.matmul(ps_o, wglob_sb, agg, start=True,
                                     stop=True)
                    osb = wp.tile([128, 512], F32, tag="osb")
                    nc.scalar.activation(out=osb, in_=ps_o, func=AF.Relu,
                                         bias=bglob_sb, scale=1.0)
                    nc.sync.dma_start(out=outp.ap()[:, tcols], in_=osb)
    nc.compile()
    return nc


_NC = None


def _get_nc():
    global _NC
    if _NC is None:
        _NC = _build_kernel()
    return _NC


def _prep_core(core, x, pos, neigh, Wc, Wn, Wr, wd, W_att, W_glob, b_rel, b_glob):
    b = core // 2
    half = core % 2
    P0 = half * NP
    nb = neigh[b][P0:P0 + NP].astype(np.int64)      # [NP, K]
    xb = x[b]                                        # [64, N] f32
    posb = pos[b]                                    # [N, 3] f32

    # token table: token n -> partition n%128, rank n//128 (128 bf16 words)
    tok = np.zeros((N, 128), dtype=BF)
    tok[:, 0:64] = xb.T.astype(BF)
    tok[:, 64:67] = posb.astype(BF)
    xtab = np.ascontiguousarray(
        tok.reshape(128, 128, 128).transpose(1, 0, 2).reshape(128, 16384))

    # gather indices: tile t chunk cc covers g cols [cc*512,(cc+1)*512) =
    # k-slab cc of the tile; col i -> nb[t*512+i, cc]; wrapped in 16
    # partitions (idx i at [i%16, i//16]), replicated to the 8 core groups
    gidx = np.zeros((128, 8192), np.int16)
    i = np.arange(512)
    for t in range(NT):
        for cc in range(K):
            nidx = nb[t * TC + i, cc].astype(np.int16)
            blk = nidx.reshape(32, 16).T           # [16, 32]
            gidx[:, t * 512 + cc * 32:t * 512 + (cc + 1) * 32] = \
                np.tile(blk, (8, 1))

    posc = np.zeros((4, NP), dtype=BF)
    posc[0:3] = posb[P0:P0 + NP].T.astype(BF)

    wcen_h = np.zeros((128, 64), dtype=BF)
    wcen_h[64:67] = (Wc - Wr).astype(BF)
    wnd_h = np.zeros((128, 64), dtype=BF)
    wnd_h[64:67] = (Wn + Wr).astype(BF)
    wnd_h[67] = wd[0].astype(BF)
    seld_h = np.zeros((128, 128), dtype=BF)
    for u in range(GC):
        seld_h[64:67, 32 * u + u] = 1.0

    perm = (np.arange(128) + 64) % 128
    brel_h = np.zeros((128, 1), np.float32)
    brel_h[64:128, 0] = b_rel

    return {
        "xtab": xtab, "gidx": gidx, "posc": posc,
        "wcen": wcen_h, "wnd": wnd_h, "seld": seld_h,
        "wattsw": W_att[np.ix_(perm, perm)].astype(BF),
        "pswap": np.roll(np.eye(128, dtype=np.float32), 64, axis=0).astype(BF),
        "wglob": W_glob.astype(BF),
        "brel": brel_h,
        "bglob": b_glob.reshape(128, 1).astype(np.float32),
    }


def kernel(x, pos, neigh_idx, W_rel, b_rel, W_att, W_glob, b_glob, **kw):
    x = np.ascontiguousarray(np.asarray(x, dtype=np.float32))
    pos = np.ascontiguousarray(np.asarray(pos, dtype=np.float32))
    neigh = np.asarray(neigh_idx)
    W_rel = np.asarray(W_rel, dtype=np.float32)
    W_att = np.asarray(W_att, dtype=np.float32)
    W_glob = np.asarray(W_glob, dtype=np.float32)
    b_rel = np.asarray(b_rel, dtype=np.float32)
    b_glob = np.asarray(b_glob, dtype=np.float32)
    Wc, Wn, Wr, wd = W_rel[0:3], W_rel[3:6], W_rel[6:9], W_rel[9:10]

    nc = _get_nc()
    in_maps = [
        _prep_core(core, x, pos, neigh, Wc, Wn, Wr, wd, W_att, W_glob, b_rel, b_glob)
        for core in range(8)
    ]
    res = run_bass_kernel_spmd(nc, in_maps, core_ids=list(range(8)))
    out = np.zeros((B, C_OUT, N), np.float32)
    for core in range(8):
        b = core // 2
        P0 = (core % 2) * NP
        out[b, :, P0:P0 + NP] = res.results[core]["outp"]
    return out


# revision 6
# speedup vs baseline: 1.9715x; 1.9715x over previous
"""Trainium2 Bass kernel for the dense RandLA-Net block.

Reference computation (per batch b, point n, K=16 neighbors):
    enc   = [center(3), npos(3), rel(3), dist(1)]            # 10 dims
    rp    = relu(enc @ W_rel + b_rel)                        # 64
    f     = [rp, nfeat]                                      # 128
    att   = softmax_k(f @ W_att)                             # 128
    agg   = sum_k f * att                                    # 128
    out   = relu(agg @ W_glob + b_glob)                      # 128

Sharding: 8 cores = 4 batches x 2 point-halves (8192 points/core).

Host prep lays the per-(point, neighbor) token streams out in the
column order the kernel consumes (tile of 512 points x 16 k-slabs):
xg holds the 64 gathered feature channels, pgd the gathered neighbor
position and the center->neighbor distance.  On device, each tile
streams both into SBUF partitions 0:64 / 64:68 and runs the fused
pipeline per 512-column k-slab: rp is a 4-row matmul [npos;dist] plus
a 3-row center matmul folded via PSUM accumulation (rel is
algebraically folded:  Wc*center + Wn*npos + Wr*(npos-center) =
(Wc-Wr)*center + (Wn+Wr)*npos), then relu(+bias) lands rp on
partitions 64:128 next to nfeat, one 128x128 matmul produces the
attention scores, exp + multiply build the softmax numerator terms,
and two PSUM-accumulated matmuls against an un-permuting identity sum
over the 16 k-slabs.  The whole pipeline runs in the "swapped"
channel layout f = [nfeat(0:64); rp(64:128)], handled by a permuted
W_att and the roll-by-64 accumulation identity.
"""

import os
import sys

import numpy as np

sys.path.insert(0, "/opt/trn_rl_repo")

import ml_dtypes

import concourse.bass as bass
import concourse.tile as tile
from concourse import mybir, bacc
from concourse.bass_utils import run_bass_kernel_spmd

F32 = mybir.dt.float32
BF16 = mybir.dt.bfloat16
AF = mybir.ActivationFunctionType
OP = mybir.AluOpType
BF = ml_dtypes.bfloat16

B, C_IN, N, K = 4, 64, 16384, 16
D_REL, C_MID, C_OUT = 64, 128, 128
NP = N // 2            # points per core
PK = NP * K            # pair columns per core (131072)
NT = 16                # tiles (= point blocks of 512)
TC = 512               # points per tile
LT = TC * K            # 8192 cols per tile


def _build_kernel():
    nc = bacc.Bacc("TRN2", target_bir_lowering=False)

    xg = nc.dram_tensor("xg", [64, PK], BF16, kind="ExternalInput")
    pgd = nc.dram_tensor("pgd", [4, PK], BF16, kind="ExternalInput")
    posc = nc.dram_tensor("posc", [4, NP], BF16, kind="ExternalInput")
    wcen = nc.dram_tensor("wcen", [128, 64], BF16, kind="ExternalInput")
    wnd = nc.dram_tensor("wnd", [128, 64], BF16, kind="ExternalInput")
    wattsw = nc.dram_tensor("wattsw", [128, 128], BF16, kind="ExternalInput")
    pswap = nc.dram_tensor("pswap", [128, 128], BF16, kind="ExternalInput")
    wglob = nc.dram_tensor("wglob", [128, 128], BF16, kind="ExternalInput")
    brel = nc.dram_tensor("brel", [128, 1], F32, kind="ExternalInput")
    bglob = nc.dram_tensor("bglob", [128, 1], F32, kind="ExternalInput")
    outp = nc.dram_tensor("outp", [128, NP], F32, kind="ExternalOutput")

    with tile.TileContext(nc) as tc:
        with tc.tile_pool(name="persist", bufs=1) as pp:
            pos_sb = pp.tile([128, NP], BF16)
            wcen_sb = pp.tile([128, 64], BF16)
            wnd_sb = pp.tile([128, 64], BF16)
            wattsw_sb = pp.tile([128, 128], BF16)
            pswap_sb = pp.tile([128, 128], BF16)
            wglob_sb = pp.tile([128, 128], BF16)
            brel_sb = pp.tile([128, 1], F32)
            bglob_sb = pp.tile([128, 1], F32)
            nc.sync.dma_start(out=pos_sb[64:68, :], in_=posc.ap())
            nc.sync.dma_start(out=wcen_sb, in_=wcen.ap())
            nc.sync.dma_start(out=wnd_sb, in_=wnd.ap())
            nc.sync.dma_start(out=wattsw_sb, in_=wattsw.ap())
            nc.sync.dma_start(out=pswap_sb, in_=pswap.ap())
            nc.sync.dma_start(out=wglob_sb, in_=wglob.ap())
            nc.sync.dma_start(out=brel_sb, in_=brel.ap())
            nc.sync.dma_start(out=bglob_sb, in_=bglob.ap())

            with tc.tile_pool(name="g", bufs=3) as gp, \
                 tc.tile_pool(name="work", bufs=2) as wp, \
                 tc.tile_pool(name="accps", bufs=1, space="PSUM") as psa, \
                 tc.tile_pool(name="mps", bufs=2, space="PSUM") as mps:
                for t in range(NT):
                    tcols = slice(t * TC, (t + 1) * TC)
                    scols = slice(t * LT, (t + 1) * LT)
                    g = gp.tile([128, LT], BF16, tag="g")
                    nc.sync.dma_start(out=g[0:64, :], in_=xg.ap()[:, scols])
                    nc.sync.dma_start(out=g[64:68, :], in_=pgd.ap()[:, scols])
                    ps_den = psa.tile([128, 512], F32, tag="den")
                    ps_num = psa.tile([128, 512], F32, tag="num")
                    for cc in range(16):
                        ccols = slice(cc * 512, (cc + 1) * 512)
                        ps_rp = mps.tile([128, 512], F32, tag="rp")
                        nc.tensor.matmul(ps_rp[64:128, :],
                                         wcen_sb[64:67, :],
                                         pos_sb[64:67, tcols],
                                         start=True, stop=False,
                                         tile_position=(64, 64),
                                         skip_group_check=True)
                        nc.tensor.matmul(ps_rp[64:128, :],
                                         wnd_sb[64:68, :],
                                         g[64:68, ccols],
                                         start=False, stop=True,
                                         tile_position=(64, 64),
                                         skip_group_check=True)
                        if cc % 2 == 0:
                            nc.scalar.activation(out=g[64:128, ccols],
                                                 in_=ps_rp[64:128, :],
                                                 func=AF.Relu,
                                                 bias=brel_sb[64:128, :],
                                                 scale=1.0)
                        else:
                            nc.vector.tensor_scalar(out=g[64:128, ccols],
                                                    in0=ps_rp[64:128, :],
                                                    scalar1=brel_sb[64:128, :],
                                                    scalar2=0.0,
                                                    op0=OP.add, op1=OP.max)
                        ps_s = mps.tile([128, 512], F32, tag="sc")
                        nc.tensor.matmul(ps_s, wattsw_sb, g[:, ccols],
                                         start=True, stop=True)
                        eu = wp.tile([128, 1024], BF16, tag="eu")
                        nc.scalar.activation(out=eu[:, 0:512], in_=ps_s,
                                             func=AF.Exp)
                        nc.vector.tensor_mul(eu[:, 512:1024], g[:, ccols],
                                             eu[:, 0:512])
                        nc.tensor.matmul(ps_den, pswap_sb, eu[:, 0:512],
                                         start=(cc == 0), stop=(cc == 15),
                                         skip_group_check=True)
                        nc.tensor.matmul(ps_num, pswap_sb, eu[:, 512:1024],
                                         start=(cc == 0), stop=(cc == 15),
                                         skip_group_check=True)
                    rcp = wp.tile([128, 512], F32, tag="rcp")
                    nc.vector.reciprocal(rcp, ps_den)
                    agg = wp.tile([128, 512], BF16, tag="agg")
                    nc.vector.tensor_mul(agg, ps_num, rcp)
                    ps_o = psa.tile([128, 512], F32, tag="den")
                    nc.tensor.matmul(ps_o, wglob_sb, agg, start=True,
                                     stop=True)
                    osb = wp.tile([128, 512], F32, tag="osb")
                    nc.scalar.activation(out=osb, in_=ps_o, func=AF.Relu,
                                         bias=bglob_sb, scale=1.0)
                    nc.sync.dma_start(out=outp.ap()[:, tcols], in_=osb)
    nc.compile()
    return nc


_NC = None


def _get_nc():
    global _NC
    if _NC is None:
        _NC = _build_kernel()
    return _NC


def _prep_core(core, x, pos, neigh, Wc, Wn, Wr, wd, W_att, W_glob, b_rel, b_glob):
    b = core // 2
    half = core % 2
    P0 = half * NP
    nb = neigh[b][P0:P0 + NP].astype(np.int64)      # [NP, K]
    xb = x[b]                                        # [64, N] f32
    posb = pos[b]                                    # [N, 3] f32

    # pair column c = t*8192 + k*512 + i -> (point n = P0 + t*512 + i, k)
    c = np.arange(PK)
    t_ = c >> 13
    k_ = (c >> 9) & 15
    i_ = c & 511
    n_ = t_ * TC + i_
    src = nb[n_, k_]                                 # neighbor point ids [PK]

    xg = xb[:, src].astype(BF)                       # [64, PK]
    npos = posb[src]                                 # [PK, 3] f32
    cen = posb[P0 + n_]                              # [PK, 3] f32
    dist = np.sqrt(((npos.astype(BF).astype(np.float32)
                     - cen.astype(BF).astype(np.float32)) ** 2).sum(1))
    pgd = np.empty((4, PK), dtype=BF)
    pgd[0:3] = npos.T.astype(BF)
    pgd[3] = dist.astype(BF)

    posc = np.zeros((4, NP), dtype=BF)
    posc[0:3] = posb[P0:P0 + NP].T.astype(BF)

    wcen_h = np.zeros((128, 64), dtype=BF)
    wcen_h[64:67] = (Wc - Wr).astype(BF)
    wnd_h = np.zeros((128, 64), dtype=BF)
    wnd_h[64:67] = (Wn + Wr).astype(BF)
    wnd_h[67] = wd[0].astype(BF)

    perm = (np.arange(128) + 64) % 128
    brel_h = np.zeros((128, 1), np.float32)
    brel_h[64:128, 0] = b_rel

    return {
        "xg": np.ascontiguousarray(xg), "pgd": pgd, "posc": posc,
        "wcen": wcen_h, "wnd": wnd_h,
        "wattsw": W_att[np.ix_(perm, perm)].astype(BF),
        "pswap": np.roll(np.eye(128, dtype=np.float32), 64, axis=0).astype(BF),
        "wglob": W_glob.astype(BF),
        "brel": brel_h,
        "bglob": b_glob.reshape(128, 1).astype(np.float32),
    }


def kernel(x, pos, neigh_idx, W_rel, b_rel, W_att, W_glob, b_glob, **kw):
    x = np.ascontiguousarray(np.asarray(x, dtype=np.float32))
    pos = np.ascontiguousarray(np.asarray(pos, dtype=np.float32))
    neigh = np.asarray(neigh_idx)
    W_rel = np.asarray(W_rel, dtype=np.float32)
    W_att = np.asarray(W_att, dtype=np.float32)
    W_glob = np.asarray(W_glob, dtype=np.float32)
    b_rel = np.asarray(b_rel, dtype=np.float32)
    b_glob = np.asarray(b_glob, dtype=np.float32)
    Wc, Wn, Wr, wd = W_rel[0:3], W_rel[3:6], W_rel[6:9], W_rel[9:10]

    nc = _get_nc()
    in_maps = [
        _prep_core(core, x, pos, neigh, Wc, Wn, Wr, wd, W_att, W_glob, b_rel, b_glob)
        for core in range(8)
    ]
    res = run_bass_kernel_spmd(nc, in_maps, core_ids=list(range(8)))
    out = np.zeros((B, C_OUT, N), np.float32)
    for core in range(8):
        b = core // 2
        P0 = (core % 2) * NP
        out[b, :, P0:P0 + NP] = res.results[core]["outp"]
    return out


# revision 7
# speedup vs baseline: 2.6903x; 1.3646x over previous
"""Trainium2 Bass kernel for the dense RandLA-Net block.

Reference computation (per batch b, point n, K=16 neighbors):
    enc   = [center(3), npos(3), rel(3), dist(1)]            # 10 dims
    rp    = relu(enc @ W_rel + b_rel)                        # 64
    f     = [rp, nfeat]                                      # 128
    att   = softmax_k(f @ W_att)                             # 128
    agg   = sum_k f * att                                    # 128
    out   = relu(agg @ W_glob + b_glob)                      # 128

Sharding: 8 cores = 4 batches x 2 point-halves (8192 points/core).

Host prep lays the per-(point, neighbor) token streams out in the
column order the kernel consumes (tile of 512 points x 16 k-slabs):
xg holds the 64 gathered feature channels, pgd the gathered neighbor
position and the center->neighbor distance.  On device, each tile
streams both into SBUF partitions 0:64 / 64:68 and runs the fused
pipeline per 512-column k-slab: rp is a 4-row matmul [npos;dist] plus
a 3-row center matmul folded via PSUM accumulation (rel is
algebraically folded:  Wc*center + Wn*npos + Wr*(npos-center) =
(Wc-Wr)*center + (Wn+Wr)*npos), then relu(+bias) lands rp on
partitions 64:128 next to nfeat, one 128x128 matmul produces the
attention scores, exp + multiply build the softmax numerator terms,
and two PSUM-accumulated matmuls against an un-permuting identity sum
over the 16 k-slabs.  The whole pipeline runs in the "swapped"
channel layout f = [nfeat(0:64); rp(64:128)], handled by a permuted
W_att and the roll-by-64 accumulation identity.
"""

import os
import sys

import numpy as np

sys.path.insert(0, "/opt/trn_rl_repo")

import ml_dtypes

import concourse.bass as bass
import concourse.tile as tile
from concourse import mybir, bacc
from concourse.bass_utils import run_bass_kernel_spmd

F32 = mybir.dt.float32
BF16 = mybir.dt.bfloat16
AF = mybir.ActivationFunctionType
OP = mybir.AluOpType
BF = ml_dtypes.bfloat16

B, C_IN, N, K = 4, 64, 16384, 16
D_REL, C_MID, C_OUT = 64, 128, 128
NP = N // 2            # points per core
PK = NP * K            # pair columns per core (131072)
NT = 16                # tiles (= point blocks of 512)
TC = 512               # points per tile
LT = TC * K            # 8192 cols per tile


def _build_kernel():
    nc = bacc.Bacc("TRN2", target_bir_lowering=False)

    xg = nc.dram_tensor("xg", [64, PK], BF16, kind="ExternalInput")
    pgd = nc.dram_tensor("pgd", [8, PK], BF16, kind="ExternalInput")
    wnd = nc.dram_tensor("wnd", [128, 64], BF16, kind="ExternalInput")
    wattsw = nc.dram_tensor("wattsw", [128, 128], BF16, kind="ExternalInput")
    pswap = nc.dram_tensor("pswap", [128, 128], BF16, kind="ExternalInput")
    wglob = nc.dram_tensor("wglob", [128, 128], BF16, kind="ExternalInput")
    bglob = nc.dram_tensor("bglob", [128, 1], F32, kind="ExternalInput")
    outp = nc.dram_tensor("outp", [128, NP], F32, kind="ExternalOutput")

    with tile.TileContext(nc) as tc:
        with tc.tile_pool(name="persist", bufs=1) as pp:
            wnd_sb = pp.tile([128, 64], BF16)
            wattsw_sb = pp.tile([128, 128], BF16)
            pswap_sb = pp.tile([128, 128], BF16)
            wglob_sb = pp.tile([128, 128], BF16)
            bglob_sb = pp.tile([128, 1], F32)
            nc.sync.dma_start(out=wnd_sb, in_=wnd.ap())
            nc.sync.dma_start(out=wattsw_sb, in_=wattsw.ap())
            nc.sync.dma_start(out=pswap_sb, in_=pswap.ap())
            nc.sync.dma_start(out=wglob_sb, in_=wglob.ap())
            nc.sync.dma_start(out=bglob_sb, in_=bglob.ap())

            with tc.tile_pool(name="g", bufs=3) as gp, \
                 tc.tile_pool(name="work", bufs=2) as wp, \
                 tc.tile_pool(name="accps", bufs=1, space="PSUM") as psa, \
                 tc.tile_pool(name="mps", bufs=2, space="PSUM") as mps:
                for t in range(NT):
                    tcols = slice(t * TC, (t + 1) * TC)
                    scols = slice(t * LT, (t + 1) * LT)
                    g = gp.tile([128, LT], BF16, tag="g")
                    nc.sync.dma_start(out=g[0:64, :], in_=xg.ap()[:, scols])
                    nc.sync.dma_start(out=g[64:72, :], in_=pgd.ap()[:, scols])
                    ps_den = psa.tile([128, 512], F32, tag="den")
                    ps_num = psa.tile([128, 512], F32, tag="num")
                    eu_prev = None
                    for cc in range(16):
                        ccols = slice(cc * 512, (cc + 1) * 512)
                        ps_rp = mps.tile([128, 512], F32, tag="rp")
                        nc.tensor.matmul(ps_rp[64:128, :],
                                         wnd_sb[64:72, :],
                                         g[64:72, ccols],
                                         start=True, stop=True,
                                         tile_position=(64, 64),
                                         skip_group_check=True)
                        if cc % 2 == 0:
                            nc.scalar.activation(out=g[64:128, ccols],
                                                 in_=ps_rp[64:128, :],
                                                 func=AF.Relu)
                        else:
                            nc.vector.tensor_scalar_max(out=g[64:128, ccols],
                                                        in0=ps_rp[64:128, :],
                                                        scalar1=0.0)
                        ps_s = mps.tile([128, 512], F32, tag="sc")
                        nc.tensor.matmul(ps_s, wattsw_sb, g[:, ccols],
                                         start=True, stop=True)
                        eu = wp.tile([128, 1024], BF16, tag="eu")
                        nc.scalar.activation(out=eu[:, 0:512], in_=ps_s,
                                             func=AF.Exp)
                        nc.vector.tensor_mul(eu[:, 512:1024], g[:, ccols],
                                             eu[:, 0:512])
                        if cc % 2 == 1:
                            eup = wp.tile([128, 1024], BF16, tag="eup")
                            nc.vector.tensor_add(eup, eu_prev, eu)
                            nc.tensor.matmul(ps_den, pswap_sb, eup[:, 0:512],
                                             start=(cc == 1), stop=(cc == 15),
                                             skip_group_check=True)
                            nc.tensor.matmul(ps_num, pswap_sb,
                                             eup[:, 512:1024],
                                             start=(cc == 1), stop=(cc == 15),
                                             skip_group_check=True)
                        eu_prev = eu
                    rcp = wp.tile([128, 512], F32, tag="rcp")
                    nc.vector.reciprocal_approx_fast(rcp, ps_den)
                    agg = wp.tile([128, 512], BF16, tag="agg")
                    nc.vector.tensor_mul(agg, ps_num, rcp)
                    ps_o = psa.tile([128, 512], F32, tag="den")
                    nc.tensor.matmul(ps_o, wglob_sb, agg, start=True,
                                     stop=True)
                    osb = wp.tile([128, 512], F32, tag="osb")
                    nc.scalar.activation(out=osb, in_=ps_o, func=AF.Relu,
                                         bias=bglob_sb, scale=1.0)
                    nc.sync.dma_start(out=outp.ap()[:, tcols], in_=osb)
    nc.compile()
    return nc


_NC = None


def _get_nc():
    global _NC
    if _NC is None:
        _NC = _build_kernel()
    return _NC


def _prep_core(core, x, pos, neigh, Wc, Wn, Wr, wd, W_att, W_glob, b_rel, b_glob):
    b = core // 2
    half = core % 2
    P0 = half * NP
    nb = neigh[b][P0:P0 + NP].astype(np.int64)      # [NP, K]
    xb = x[b]                                        # [64, N] f32
    posb = pos[b]                                    # [N, 3] f32

    # pair column c = t*8192 + k*512 + i -> (point n = P0 + t*512 + i, k)
    c = np.arange(PK)
    t_ = c >> 13
    k_ = (c >> 9) & 15
    i_ = c & 511
    n_ = t_ * TC + i_
    src = nb[n_, k_]                                 # neighbor point ids [PK]

    xg = xb[:, src].astype(BF)                       # [64, PK]
    npos = posb[src]                                 # [PK, 3] f32
    cen = posb[P0 + n_]                              # [PK, 3] f32
    dist = np.sqrt(((npos.astype(BF).astype(np.float32)
                     - cen.astype(BF).astype(np.float32)) ** 2).sum(1))
    pgd = np.empty((8, PK), dtype=BF)
    pgd[0:3] = npos.T.astype(BF)
    pgd[3] = dist.astype(BF)
    pgd[4:7] = cen.T.astype(BF)
    pgd[7] = 1.0

    wnd_h = np.zeros((128, 64), dtype=BF)
    wnd_h[64:67] = (Wn + Wr).astype(BF)
    wnd_h[67] = wd[0].astype(BF)
    wnd_h[68:71] = (Wc - Wr).astype(BF)
    wnd_h[71] = b_rel.astype(BF)

    perm = (np.arange(128) + 64) % 128

    return {
        "xg": np.ascontiguousarray(xg), "pgd": pgd,
        "wnd": wnd_h,
        "wattsw": W_att[np.ix_(perm, perm)].astype(BF),
        "pswap": np.roll(np.eye(128, dtype=np.float32), 64, axis=0).astype(BF),
        "wglob": W_glob.astype(BF),
        "bglob": b_glob.reshape(128, 1).astype(np.float32),
    }


def kernel(x, pos, neigh_idx, W_rel, b_rel, W_att, W_glob, b_glob, **kw):
    x = np.ascontiguousarray(np.asarray(x, dtype=np.float32))
    pos = np.ascontiguousarray(np.asarray(pos, dtype=np.float32))
    neigh = np.asarray(neigh_idx)
    W_rel = np.asarray(W_rel, dtype=np.float32)
    W_att = np.asarray(W_att, dtype=np.float32)
    W_glob = np.asarray(W_glob, dtype=np.float32)
    b_rel = np.asarray(b_rel, dtype=np.float32)
    b_glob = np.asarray(b_glob, dtype=np.float32)
    Wc, Wn, Wr, wd = W_rel[0:3], W_rel[3:6], W_rel[6:9], W_rel[9:10]

    nc = _get_nc()
    in_maps = [
        _prep_core(core, x, pos, neigh, Wc, Wn, Wr, wd, W_att, W_glob, b_rel, b_glob)
        for core in range(8)
    ]
    res = run_bass_kernel_spmd(nc, in_maps, core_ids=list(range(8)))
    out = np.zeros((B, C_OUT, N), np.float32)
    for core in range(8):
        b = core // 2
        P0 = (core % 2) * NP
        out[b, :, P0:P0 + NP] = res.results[core]["outp"]
    return out
